# revision 1
# baseline (speedup 1.0000x reference)
"""Causal multi-head attention (B=128, T=256, C=384, H=6, Dh=64) on 8 TRN2
NeuronCores, data-parallel over batch (16 batches per core, no collectives).

Layout strategy per core (v4 — software-pipelined):
  - host pre-transposes x to xT [b, C, T] and casts activations/weights to bf16
  - QT/KT computed as [D, T] (Dh on partitions) so scores = QT_h.T @ KT_h needs
    no on-chip transpose of Q/K; V computed as [T, D]
  - scores per head land in fp32 PSUM as three 128-col blocks
    [tq0ts0 | tq1ts1 | tq1ts0]
  - exp on Scalar; fused causal-mask-multiply + row-sum via one DVE
    scalar_tensor_tensor per (sub, tq-block); reciprocal + normalize on DVE
  - P transposed on the PE (bf16) into one packed PSUM bank per pair
  - AV col-packs the head pair via tile_position; output projection consumes
    OT [D, T] directly; y stored bf16 (host casts back to fp32)
  - group g+1's QK/V projection matmuls are emitted interleaved with group
    g's attention pairs so the PE always has filler work during softmax and
    the HAM clock gate stays warm
  - dedicated PSUM pools (proj 2 / scores 2 / transposes 2 / AV-out 2 banks)
"""

import sys

sys.path.insert(0, "/opt/trn_rl_repo")

import numpy as np
import ml_dtypes

import concourse.bass as bass
import concourse.tile as tile
from concourse import mybir
from concourse.bass_utils import run_bass_kernel_spmd
from concourse.masks import make_identity

def split_multi_waits(nc):
    """This walrus build accepts at most one sync-wait command per
    instruction; hoist extra waits into standalone InstEventSemaphore
    instructions on the same engine queue (queue waits run in order before
    the original instruction, so semantics are preserved)."""
    ctr = [0]

    def mk(engine, wait):
        ctr[0] += 1
        return mybir.InstEventSemaphore(
            name=f"WSPLIT-{ctr[0]}",
            engine=engine,
            ins=[],
            outs=[],
            sync_info=mybir.SyncInfo(on_wait=[wait], on_update=[]),
        )

    for f in nc.m.functions:
        for blk in f.blocks:
            insts = blk.instructions
            out = []
            for inst in insts:
                si = inst.sync_info
                if si is not None and len(si.on_wait) > 1:
                    waits = list(si.on_wait)
                    for w in waits[:-1]:
                        out.append(mk(inst.engine, w))
                    inst.sync_info = mybir.SyncInfo(
                        on_wait=[waits[-1]], on_update=list(si.on_update)
                    )
                out.append(inst)
            insts[:] = out
    return nc


N_CORES = 8
B, T, C = 128, 256, 384
H, DH = 6, 64
BL = B // N_CORES  # batches per core
GB = 2  # batches per projection group (N = GB*T = 512 <= one PSUM bank fp32)
NG = BL // GB
BF16 = mybir.dt.bfloat16
FP32 = mybir.dt.float32
AFT = mybir.ActivationFunctionType
MUL = mybir.AluOpType.mult
SCALE = DH**-0.5  # 0.125


def build_kernel() -> bass.Bass:
    nc = bass.Bass()
    xT = nc.dram_tensor("xT", [BL, C, T], BF16, kind="ExternalInput")
    wqt = nc.dram_tensor("wqt", [C, C], BF16, kind="ExternalInput")  # Wq.T [C, D]
    wkt = nc.dram_tensor("wkt", [C, C], BF16, kind="ExternalInput")
    wvt = nc.dram_tensor("wvt", [C, C], BF16, kind="ExternalInput")
    wot = nc.dram_tensor("wot", [C, C], BF16, kind="ExternalInput")  # Wo.T [D, C]
    y = nc.dram_tensor("y", [BL, T, C], BF16, kind="ExternalOutput")

    with tile.TileContext(nc) as tc:
        with (
            tc.tile_pool(name="const", bufs=1) as const,
            tc.tile_pool(name="xp", bufs=2) as xp,
            tc.tile_pool(name="qkv", bufs=2) as qkv,
            tc.tile_pool(name="pp", bufs=3) as pp,
            tc.tile_pool(name="st", bufs=3) as st,
            tc.tile_pool(name="ptsb", bufs=3) as ptsb,
            tc.tile_pool(name="otp", bufs=2) as otp,
            tc.tile_pool(name="yp", bufs=3) as yp,
            tc.tile_pool(name="psProj", bufs=2, space="PSUM") as psProj,
            tc.tile_pool(name="psSc", bufs=5, space="PSUM") as psSc,
            tc.tile_pool(name="psPo", bufs=1, space="PSUM") as psPo,
        ):
            # prefetch x for group 0 ahead of the (larger) weight DMAs so the
            # first projections start ASAP
            xt0 = xp.tile([128, 3, GB, T], BF16, name="xt_g0")
            for bi in range(GB):
                nc.sync.dma_start(
                    out=xt0[:, :, bi, :],
                    in_=xT[bi].rearrange("(k p) t -> p k t", p=128),
                )
            ident = const.tile([128, 128], BF16)
            make_identity(nc, ident)
            # tiny dummy exp: forces the ACT exp-table load during the DMA
            # wait instead of on the first real softmax
            dummy = const.tile([128, 2], FP32)
            nc.scalar.activation(dummy, ident[:, 0:2], AFT.Exp, scale=1.0)
            # multiplicative causal masks (bf16), applied post-exp inside the
            # fused mask*P + row-sum DVE op. Block order per sub is
            # [tq0ts0 | tq1ts0 | tq1ts1]: blocks 0,2 are lower-triangle,
            # block 1 is all-ones. mtri covers blocks 1:3 (the tq1 row);
            # mtri[:,1,:] is reused for block 0.
            mtri = const.tile([128, 2, 128], BF16)
            nc.gpsimd.memset(mtri, 1.0)
            nc.gpsimd.affine_select(
                out=mtri[:, 1, :], in_=mtri[:, 1, :],
                compare_op=mybir.AluOpType.is_ge,
                fill=0.0, base=0, pattern=[[-1, 128]], channel_multiplier=1,
            )

            w_sb = {}
            for name, dram in (("wq", wqt), ("wk", wkt), ("wv", wvt), ("wo", wot)):
                w = const.tile([128, 3, C], BF16, tag=name)
                nc.sync.dma_start(out=w, in_=dram.rearrange("(k p) d -> p k d", p=128))
                w_sb[name] = w

            def load_group(g, xt=None):
                """DMA xT for group g, allocate qt/kt/v tiles."""
                if xt is None:
                    xt = xp.tile([128, 3, GB, T], BF16, name=f"xt{g}")
                    for bi in range(GB):
                        nc.sync.dma_start(
                            out=xt[:, :, bi, :],
                            in_=xT[g * GB + bi].rearrange(
                                "(k p) t -> p k t", p=128
                            ),
                        )
                qt = qkv.tile([128, 3, GB, T], BF16, tag="qt", name=f"qt{g}")
                kt = qkv.tile([128, 3, GB, T], BF16, tag="kt", name=f"kt{g}")
                vs = [
                    qkv.tile([128, 2, C], BF16, tag=f"v{bi}", name=f"v{g}_{bi}")
                    for bi in range(GB)
                ]
                return xt, qt, kt, vs

            def proj_emitters(xt, qt, kt, vs):
                """Closures each emitting one PSUM-chunk of the QK/V
                projections (3 accumulating matmuls + 1 evacuation). Ordered
                so the consumers' dependencies resolve earliest-first:
                qk chunk d feeds attention pair d; v[bi] feeds batch bi."""
                def qk_em(dst, wname, d):
                    def em():
                        ps = psProj.tile([128, GB * T], FP32, tag="proj",
                                         name="psqk")
                        for k in range(3):
                            nc.tensor.matmul(
                                ps,
                                lhsT=w_sb[wname][:, k, d * 128:(d + 1) * 128],
                                rhs=xt[:, k, :, :],
                                start=(k == 0), stop=(k == 2),
                            )
                        nc.scalar.copy(dst[:, d, :, :], ps)
                    return em

                def v_em(bi, t2):
                    def em():
                        ps = psProj.tile([128, GB * T], FP32, tag="proj",
                                         name="psv")
                        for k in range(3):
                            nc.tensor.matmul(
                                ps[:, 0:C],
                                lhsT=xt[:, k, bi, t2 * 128:(t2 + 1) * 128],
                                rhs=w_sb["wv"][:, k, :],
                                start=(k == 0), stop=(k == 2),
                            )
                        nc.scalar.copy(vs[bi][:, t2, :], ps[:, 0:C])
                    return em

                return [
                    qk_em(qt, "wq", 0), qk_em(kt, "wk", 0),
                    v_em(0, 0), v_em(0, 1),
                    qk_em(qt, "wq", 1), qk_em(kt, "wk", 1),
                    qk_em(qt, "wq", 2), qk_em(kt, "wk", 2),
                    v_em(1, 0), v_em(1, 1),
                ]

            po_state = {"tile": None, "idx": 0}

            def att_pair(qt, kt, v, bi, pair, ot, filler=None):
                # ---- scores: fp32 psum, one bank per sub ----
                # col blocks per sub: 0 = (tq0,ts0), 1 = (tq1,ts0), 2 = (tq1,ts1)
                sc = [
                    psSc.tile([128, 3, 128], FP32, tag="sc", name=f"sc{s}")
                    for s in range(2)
                ]
                for s in range(2):
                    doff = s * 64
                    nc.tensor.matmul(
                        sc[s][:, 0, :],
                        lhsT=qt[doff:doff + 64, pair, bi, 0:128],
                        rhs=kt[doff:doff + 64, pair, bi, 0:128],
                        start=True, stop=True,
                    )
                for s in range(2):
                    doff = s * 64
                    nc.tensor.matmul(
                        sc[s][:, 1:3, :],
                        lhsT=qt[doff:doff + 64, pair, bi, 128:256],
                        rhs=kt[doff:doff + 64, pair, bi, 0:256],
                        start=True, stop=True,
                    )
                # ---- exp, then fused causal-mask * P + row sums ----
                p = pp.tile([128, 2, 3, 128], BF16, tag="p")
                sums = st.tile([128, 4], FP32, tag="sums")
                rs = st.tile([128, 4], FP32, tag="rs")
                for s in range(2):
                    nc.scalar.activation(
                        p[:, s, :, :], sc[s], AFT.Exp, scale=SCALE
                    )
                    nc.vector.scalar_tensor_tensor(
                        out=p[:, s, 0, :], in0=p[:, s, 0, :],
                        scalar=1.0, in1=mtri[:, 1, :],
                        op0=MUL, op1=MUL,
                        accum_out=sums[:, 2 * s:2 * s + 1],
                    )
                    nc.vector.scalar_tensor_tensor(
                        out=p[:, s, 1:3, :], in0=p[:, s, 1:3, :],
                        scalar=1.0, in1=mtri,
                        op0=MUL, op1=MUL,
                        accum_out=sums[:, 2 * s + 1:2 * s + 2],
                    )
                nc.vector.reciprocal(rs, sums)
                # ---- normalize P (DVE per-partition scalar mul) ----
                for s in range(2):
                    nc.vector.tensor_scalar_mul(
                        p[:, s, 0, :], p[:, s, 0, :], rs[:, 2 * s:2 * s + 1]
                    )
                    nc.vector.tensor_scalar_mul(
                        p[:, s, 1:3, :], p[:, s, 1:3, :],
                        rs[:, 2 * s + 1:2 * s + 2],
                    )
                if filler is not None:
                    filler()
                # ---- transpose P via the DMA XBAR (SBUF->SBUF), one DMA
                # per sub; out[p, j, q] = in[q, j*128+p], i.e. piece j holds
                # the transpose of P's j-th 128-col block ----
                pt = ptsb.tile([128, 2, 3, 128], BF16, tag="ptsb")
                for s in range(2):
                    nc.sync.dma_start(
                        out=pt[:, s, :, :], in_=p[:, s, :, :], transpose=True
                    )
                # ---- AV: col-packed head pair; two pairs share one PSUM
                # bank ([128,512] fp32), alternating halves ----
                if po_state["idx"] % 2 == 0:
                    po_state["tile"] = psPo.tile(
                        [128, 2, T], FP32, tag="po", name="po2"
                    )
                po = po_state["tile"][:, po_state["idx"] % 2, :]
                po_state["idx"] += 1
                for mm in range(3):
                    for s in range(2):
                        h = 2 * pair + s
                        doff = s * 64
                        vsl = lambda ts: v[:, ts, h * 64:(h + 1) * 64]
                        if mm == 0:
                            nc.tensor.matmul(
                                po[doff:doff + 64, 0:128],
                                lhsT=vsl(0), rhs=pt[:, s, 0, :],
                                start=True, stop=True,
                                tile_position=(0, doff),
                            )
                        else:
                            nc.tensor.matmul(
                                po[doff:doff + 64, 128:256],
                                lhsT=vsl(mm - 1), rhs=pt[:, s, mm, :],
                                start=(mm == 1), stop=(mm == 2),
                                tile_position=(0, doff),
                            )
                if pair == 1:
                    nc.scalar.copy(ot[:, pair, :], po)
                else:
                    nc.vector.tensor_copy(ot[:, pair, :], po)

            def emit_y(b, ot):
                ys = yp.tile([128, 2, C], BF16)
                for t2 in range(2):
                    ps = psProj.tile([128, GB * T], FP32, tag="proj", name="psy")
                    for k in range(3):
                        nc.tensor.matmul(
                            ps[:, 0:C],
                            lhsT=ot[:, k, t2 * 128:(t2 + 1) * 128],
                            rhs=w_sb["wo"][:, k, :],
                            start=(k == 0), stop=(k == 2),
                        )
                    nc.scalar.copy(ys[:, t2, :], ps[:, 0:C])
                nc.sync.dma_start(
                    out=y[b].rearrange("(t2 p) c -> p t2 c", p=128), in_=ys
                )

            # ---- prologue: only the chunks pair (b0, p0) needs up front;
            # the rest of group 0's projections become its own filler ----
            cur = load_group(0, xt=xt0)
            g0_ems = proj_emitters(cur[0], cur[1], cur[2], cur[3])
            for em in g0_ems[:4]:
                em()
            carry = g0_ems[4:]

            for g in range(NG):
                nxt = None
                nxt_ems = list(carry)
                carry = []
                if g + 1 < NG:
                    nxt = load_group(g + 1)
                    nxt_ems += proj_emitters(nxt[0], nxt[1], nxt[2], nxt[3])
                _, qt, kt, vs = cur
                ei = [0]

                def filler(nxt_ems=nxt_ems, ei=ei):
                    # one projection chunk, emitted between softmax and
                    # transposes so the PE has work during the DVE stages
                    if ei[0] < len(nxt_ems):
                        nxt_ems[ei[0]]()
                        ei[0] += 1

                for bi in range(GB):
                    b = g * GB + bi
                    ot = otp.tile([128, 3, T], BF16)
                    for pair in range(3):
                        filler()
                        att_pair(qt, kt, vs[bi], bi, pair, ot, filler=filler)
                    emit_y(b, ot)
                while ei[0] < len(nxt_ems):
                    nxt_ems[ei[0]]()
                    ei[0] += 1
                cur = nxt
    return nc


_NC = None


def _get_nc():
    global _NC
    if _NC is None:
        _NC = split_multi_waits(build_kernel())
    return _NC


def kernel(x, Wq, Wk, Wv, Wo, _trace=False):
    bf16 = ml_dtypes.bfloat16
    wq_t = np.ascontiguousarray(Wq.T).astype(bf16)
    wk_t = np.ascontiguousarray(Wk.T).astype(bf16)
    wv_t = np.ascontiguousarray(Wv.T).astype(bf16)
    wo_t = np.ascontiguousarray(Wo.T).astype(bf16)
    in_maps = []
    for i in range(N_CORES):
        xs = x[i * BL : (i + 1) * BL]  # [BL, T, C]
        xs_t = np.ascontiguousarray(xs.transpose(0, 2, 1)).astype(bf16)
        in_maps.append(
            {"xT": xs_t, "wqt": wq_t, "wkt": wk_t, "wvt": wv_t, "wot": wo_t}
        )
    res = run_bass_kernel_spmd(
        _get_nc(), in_maps, list(range(N_CORES)), trace=_trace
    )
    out = np.concatenate([r["y"] for r in res.results], axis=0)
    if _trace:
        return out.astype(np.float32), res
    return out.astype(np.float32)



# revision 4
# speedup vs baseline: 1.6954x; 1.6954x over previous
"""Causal multi-head attention (B=128, T=256, C=384, H=6, Dh=64) on 8 TRN2
NeuronCores, data-parallel over batch (16 batches per core, no collectives).

Layout strategy per core (v5 — transposed scores, fused denominator):
  - host pre-transposes x to xT [b, C, T] and casts activations/weights to bf16
  - QT/KT computed as [D, T] (Dh on partitions); V computed as [T, H, 65]
    with a constant-1.0 65th column per head ("v_aug")
  - scores are computed TRANSPOSED: S_T[ts, tq] = K_blk.T-style matmuls with
    kt as the stationary operand, in three 128x128 blocks per sub-head
    ordered [ts0tq0 | ts1tq1 | ts0tq1] (first two need the causal mask)
  - exp on ACT (one strided call per pair covering both subs); causal mask as
    one DVE multiply over the two triangle blocks (mtriT broadcast)
  - AV: lhsT = P_T block, rhs = v_aug -> O lands [tq, d] in PSUM and the
    softmax denominator appears for free in column 64 of each 65-wide block
  - normalize = DVE reciprocal [128,4] + ONE tensor_tensor multiply with the
    reciprocal broadcast along d (per-partition = per-tq -> cheap), which also
    serves as the PSUM->SBUF evacuation (bf16 cast)
  - per batch, O [tq, 384] is DMA-XBAR-transposed (2 calls) to OT [d, tq] for
    the output projection (32 transposes total vs 96 for per-pair P^T)
  - x loads and y stores ride the GpSimd SWDGE queue so the Sync queue only
    carries weights + O-transposes
  - group g+1's QK/V projection matmuls are interleaved as PE filler during
    group g's softmax waits; y-projections are deferred by ~1 pair so the
    PE never waits on the O-transpose chain
"""

import sys

sys.path.insert(0, "/opt/trn_rl_repo")

import numpy as np
import ml_dtypes

import concourse.bass as bass
import concourse.tile as tile
from concourse import mybir
from concourse.bass_utils import run_bass_kernel_spmd

def split_multi_waits(nc):
    """This walrus build accepts at most one sync-wait command per
    instruction; hoist extra waits into standalone InstEventSemaphore
    instructions on the same engine queue (queue waits run in order before
    the original instruction, so semantics are preserved)."""
    ctr = [0]

    def mk(engine, wait):
        ctr[0] += 1
        return mybir.InstEventSemaphore(
            name=f"WSPLIT-{ctr[0]}",
            engine=engine,
            ins=[],
            outs=[],
            sync_info=mybir.SyncInfo(on_wait=[wait], on_update=[]),
        )

    for f in nc.m.functions:
        for blk in f.blocks:
            insts = blk.instructions
            out = []
            for inst in insts:
                si = inst.sync_info
                if si is not None and len(si.on_wait) > 1:
                    waits = list(si.on_wait)
                    for w in waits[:-1]:
                        out.append(mk(inst.engine, w))
                    inst.sync_info = mybir.SyncInfo(
                        on_wait=[waits[-1]], on_update=list(si.on_update)
                    )
                out.append(inst)
            insts[:] = out
    return nc


N_CORES = 8
B, T, C = 128, 256, 384
H, DH = 6, 64
BL = B // N_CORES  # batches per core
GB = 2  # batches per projection group (N = GB*T = 512 <= one PSUM bank fp32)
NG = BL // GB
BF16 = mybir.dt.bfloat16
FP32 = mybir.dt.float32
AFT = mybir.ActivationFunctionType
SCALE = DH**-0.5  # 0.125


def build_kernel() -> bass.Bass:
    nc = bass.Bass()
    xT = nc.dram_tensor("xT", [BL, C, T], BF16, kind="ExternalInput")
    wqt = nc.dram_tensor("wqt", [C, C], BF16, kind="ExternalInput")  # Wq.T [C, D]
    wkt = nc.dram_tensor("wkt", [C, C], BF16, kind="ExternalInput")
    wvt = nc.dram_tensor("wvt", [C, C], BF16, kind="ExternalInput")
    wot = nc.dram_tensor("wot", [C, C], BF16, kind="ExternalInput")  # Wo.T [D, C]
    y = nc.dram_tensor("y", [BL, T, C], BF16, kind="ExternalOutput")

    with tile.TileContext(nc) as tc:
        with (
            tc.tile_pool(name="const", bufs=1) as const,
            tc.tile_pool(name="xp", bufs=2) as xp,
            tc.tile_pool(name="qkv", bufs=2) as qkv,
            tc.tile_pool(name="pp", bufs=3) as pp,
            tc.tile_pool(name="st", bufs=3) as st,
            tc.tile_pool(name="osb", bufs=3) as osb,
            tc.tile_pool(name="otp", bufs=3) as otp,
            tc.tile_pool(name="yp", bufs=3) as yp,
            tc.tile_pool(name="psProj", bufs=2, space="PSUM") as psProj,
            tc.tile_pool(name="psSc", bufs=2, space="PSUM") as psSc,
            tc.tile_pool(name="psPo", bufs=2, space="PSUM") as psPo,
        ):
            # prefetch x for group 0 (SWDGE queue) ahead of the (larger)
            # weight DMAs (sync queue) so the first projections start ASAP
            xt0 = xp.tile([128, 3, GB, T], BF16, name="xt_g0")
            for bi in range(GB):
                nc.gpsimd.dma_start(
                    out=xt0[:, :, bi, :],
                    in_=xT[bi].rearrange("(k p) t -> p k t", p=128),
                )
            # multiplicative causal mask for TRANSPOSED scores [ts, tq]:
            # keep tq >= ts, i.e. col >= partition (upper triangle + diag)
            mtriT = const.tile([128, 128], BF16)
            nc.gpsimd.memset(mtriT, 1.0)
            nc.gpsimd.affine_select(
                out=mtriT, in_=mtriT,
                compare_op=mybir.AluOpType.is_ge,
                fill=0.0, base=0, pattern=[[1, 128]], channel_multiplier=-1,
            )
            # tiny dummy exp: forces the ACT exp-table load during the DMA
            # wait instead of on the first real softmax
            dummy = const.tile([128, 2], FP32)
            nc.scalar.activation(dummy, mtriT[:, 0:2], AFT.Exp, scale=1.0)

            w_sb = {}
            for name, dram in (("wq", wqt), ("wk", wkt), ("wv", wvt), ("wo", wot)):
                w = const.tile([128, 3, C], BF16, tag=name)
                nc.sync.dma_start(out=w, in_=dram.rearrange("(k p) d -> p k d", p=128))
                w_sb[name] = w

            def load_group(g, xt=None):
                """DMA xT for group g, allocate qt/kt/v_aug tiles."""
                if xt is None:
                    xt = xp.tile([128, 3, GB, T], BF16, name=f"xt{g}")
                    for bi in range(GB):
                        nc.gpsimd.dma_start(
                            out=xt[:, :, bi, :],
                            in_=xT[g * GB + bi].rearrange(
                                "(k p) t -> p k t", p=128
                            ),
                        )
                qt = qkv.tile([128, 3, GB, T], BF16, tag="qt", name=f"qt{g}")
                kt = qkv.tile([128, 3, GB, T], BF16, tag="kt", name=f"kt{g}")
                vs = []
                for bi in range(GB):
                    # head stride 68 (not 65) keeps every rhs slice 8B-aligned
                    v = qkv.tile(
                        [128, 2, H, 68], BF16, tag=f"v{bi}", name=f"v{g}_{bi}"
                    )
                    # constant 1.0 column 64 -> AV matmul emits the softmax
                    # denominator for free
                    nc.gpsimd.memset(v[:, :, :, 64:65], 1.0)
                    vs.append(v)
                return xt, qt, kt, vs

            def proj_emitters(xt, qt, kt, vs):
                """Closures each emitting one PSUM-chunk of the QK/V
                projections (3 accumulating matmuls + 1 evacuation). Ordered
                so the consumers' dependencies resolve earliest-first."""
                def qk_em(dst, wname, d):
                    def em():
                        ps = psProj.tile([128, GB * T], FP32, tag="proj",
                                         name="psqk")
                        for k in range(3):
                            nc.tensor.matmul(
                                ps,
                                lhsT=w_sb[wname][:, k, d * 128:(d + 1) * 128],
                                rhs=xt[:, k, :, :],
                                start=(k == 0), stop=(k == 2),
                            )
                        nc.scalar.copy(dst[:, d, :, :], ps)
                    return em

                def v_em(bi, t2):
                    def em():
                        ps = psProj.tile([128, GB * T], FP32, tag="proj",
                                         name="psv")
                        for k in range(3):
                            nc.tensor.matmul(
                                ps[:, 0:C],
                                lhsT=xt[:, k, bi, t2 * 128:(t2 + 1) * 128],
                                rhs=w_sb["wv"][:, k, :],
                                start=(k == 0), stop=(k == 2),
                            )
                        nc.vector.tensor_copy(
                            vs[bi][:, t2, :, 0:64],
                            ps[:, 0:C].rearrange("p (h j) -> p h j", j=64),
                        )
                    return em

                return [
                    qk_em(qt, "wq", 0), qk_em(kt, "wk", 0),
                    v_em(0, 0), v_em(0, 1),
                    qk_em(qt, "wq", 1), qk_em(kt, "wk", 1),
                    qk_em(qt, "wq", 2), qk_em(kt, "wk", 2),
                    v_em(1, 0), v_em(1, 1),
                ]

            def att_pair(qt, kt, v, bi, pair, o_sbt, filler=None):
                # ---- transposed scores S_T[ts, tq], fp32 psum ----
                # block order per sub: [ts0tq0 | ts1tq1 | ts0tq1]
                # (triangle blocks first so the mask is one contiguous slice)
                sc = psSc.tile([128, 2, 512], FP32, tag="sc", name="sc")
                for s in range(2):
                    doff = s * 64
                    kts = kt[doff:doff + 64, pair, bi, :]
                    qts = qt[doff:doff + 64, pair, bi, :]
                    nc.tensor.matmul(
                        sc[:, s, 0:128], lhsT=kts[:, 0:128],
                        rhs=qts[:, 0:128], start=True, stop=True,
                    )
                    nc.tensor.matmul(
                        sc[:, s, 256:384], lhsT=kts[:, 0:128],
                        rhs=qts[:, 128:256], start=True, stop=True,
                    )
                    nc.tensor.matmul(
                        sc[:, s, 128:256], lhsT=kts[:, 128:256],
                        rhs=qts[:, 128:256], start=True, stop=True,
                    )
                # ---- exp on ACT (both subs, one strided call) ----
                p_t = pp.tile([128, 2, 3, 128], BF16, tag="p")
                nc.scalar.activation(
                    p_t,
                    sc[:, :, 0:384].rearrange("p s (k c) -> p s k c", c=128),
                    AFT.Exp, scale=SCALE,
                )
                # ---- causal mask: one DVE multiply over the 2 triangle
                # blocks of both subs ----
                nc.vector.tensor_mul(
                    p_t[:, :, 0:2, :], p_t[:, :, 0:2, :],
                    mtriT[:, None, None, :].to_broadcast((128, 2, 2, 128)),
                )
                if filler is not None:
                    filler()
                # ---- AV + fused denominator: O[tq, 65] per (tqb, s) ----
                # block stride 66 fp32 = 264B keeps matmul PSUM outputs
                # 8B-aligned (PSUM cacheline)
                po = psPo.tile([128, 2, 2, 66], FP32, tag="po", name="po")
                for s in range(2):
                    h = 2 * pair + s
                    nc.tensor.matmul(
                        po[:, 0, s, 0:65], lhsT=p_t[:, s, 0, :],
                        rhs=v[:, 0, h, 0:65], start=True, stop=True,
                    )
                    nc.tensor.matmul(
                        po[:, 1, s, 0:65], lhsT=p_t[:, s, 2, :],
                        rhs=v[:, 0, h, 0:65], start=True, stop=False,
                    )
                    nc.tensor.matmul(
                        po[:, 1, s, 0:65], lhsT=p_t[:, s, 1, :],
                        rhs=v[:, 1, h, 0:65], start=False, stop=True,
                    )
                # ---- normalize: per-partition (=per-tq) reciprocal, then
                # one broadcast multiply that doubles as the PSUM->SBUF
                # evacuation ----
                rs = st.tile([128, 2, 2], FP32, tag="rs")
                nc.vector.reciprocal(rs, po[:, :, :, 64])
                out_sl = o_sbt[:, :, pair * 128:(pair + 1) * 128].rearrange(
                    "p t (s j) -> p t s j", j=64
                )
                nc.vector.tensor_mul(
                    out_sl, po[:, :, :, 0:64],
                    rs[:, :, :, None].to_broadcast((128, 2, 2, 64)),
                )

            def emit_trans(o_sbt):
                otp_t = otp.tile([128, 2, 3, 128], BF16)
                for tqb in range(2):
                    nc.sync.dma_start(
                        out=otp_t[:, tqb, :, :],
                        in_=o_sbt[:, tqb, :].rearrange("p (k c) -> p k c", c=128),
                        transpose=True,
                    )
                return otp_t

            def emit_yproj(b, otp_t):
                ys = yp.tile([128, 2, C], BF16)
                for tqb in range(2):
                    ps = psProj.tile([128, GB * T], FP32, tag="proj", name="psy")
                    for k in range(3):
                        nc.tensor.matmul(
                            ps[:, 0:C],
                            lhsT=otp_t[:, tqb, k, :],
                            rhs=w_sb["wo"][:, k, :],
                            start=(k == 0), stop=(k == 2),
                        )
                    nc.vector.tensor_copy(ys[:, tqb, :], ps[:, 0:C])
                nc.gpsimd.dma_start(
                    out=y[b].rearrange("(t2 p) c -> p t2 c", p=128), in_=ys
                )

            # ---- prologue: only the chunks pair (b0, p0) needs up front;
            # the rest of group 0's projections become its own filler ----
            cur = load_group(0, xt=xt0)
            g0_ems = proj_emitters(cur[0], cur[1], cur[2], cur[3])
            for em in g0_ems[:4]:
                em()
            carry = g0_ems[4:]
            pending_y = []

            def flush_y():
                while pending_y:
                    b, otp_t = pending_y.pop(0)
                    emit_yproj(b, otp_t)

            for g in range(NG):
                nxt = None
                nxt_ems = list(carry)
                carry = []
                if g + 1 < NG:
                    nxt = load_group(g + 1)
                    nxt_ems += proj_emitters(nxt[0], nxt[1], nxt[2], nxt[3])
                _, qt, kt, vs = cur
                ei = [0]

                def filler(nxt_ems=nxt_ems, ei=ei):
                    # one projection chunk, emitted inside the softmax wait
                    # so the PE always has independent work
                    if ei[0] < len(nxt_ems):
                        nxt_ems[ei[0]]()
                        ei[0] += 1

                for bi in range(GB):
                    b = g * GB + bi
                    o_sbt = osb.tile([128, 2, C], BF16)
                    for pair in range(3):
                        filler()
                        att_pair(qt, kt, vs[bi], bi, pair, o_sbt, filler=filler)
                        if pair == 1:
                            # y-projection of the previous batch, deferred so
                            # the PE never waits on the O-transpose chain
                            flush_y()
                    pending_y.append((b, emit_trans(o_sbt)))
                while ei[0] < len(nxt_ems):
                    nxt_ems[ei[0]]()
                    ei[0] += 1
                cur = nxt
            flush_y()
    return nc


_NC = None


def _get_nc():
    global _NC
    if _NC is None:
        _NC = split_multi_waits(build_kernel())
    return _NC


def kernel(x, Wq, Wk, Wv, Wo, _trace=False):
    bf16 = ml_dtypes.bfloat16
    wq_t = np.ascontiguousarray(Wq.T).astype(bf16)
    wk_t = np.ascontiguousarray(Wk.T).astype(bf16)
    wv_t = np.ascontiguousarray(Wv.T).astype(bf16)
    wo_t = np.ascontiguousarray(Wo.T).astype(bf16)
    in_maps = []
    for i in range(N_CORES):
        xs = x[i * BL : (i + 1) * BL]  # [BL, T, C]
        xs_t = np.ascontiguousarray(xs.transpose(0, 2, 1)).astype(bf16)
        in_maps.append(
            {"xT": xs_t, "wqt": wq_t, "wkt": wk_t, "wvt": wv_t, "wot": wo_t}
        )
    res = run_bass_kernel_spmd(
        _get_nc(), in_maps, list(range(N_CORES)), trace=_trace
    )
    out = np.concatenate([r["y"] for r in res.results], axis=0)
    if _trace:
        return out.astype(np.float32), res
    return out.astype(np.float32)


# revision 8
# speedup vs baseline: 1.7489x; 1.0316x over previous
"""Causal multi-head attention (B=128, T=256, C=384, H=6, Dh=64) on 8 TRN2
NeuronCores, data-parallel over batch (16 batches per core, no collectives).

Layout strategy per core (v5 — transposed scores, fused denominator):
  - host pre-transposes x to xT [b, C, T] and casts activations/weights to bf16
  - QT/KT computed as [D, T] (Dh on partitions); V computed as [T, H, 65]
    with a constant-1.0 65th column per head ("v_aug")
  - scores are computed TRANSPOSED: S_T[ts, tq] = K_blk.T-style matmuls with
    kt as the stationary operand, in three 128x128 blocks per sub-head
    ordered [ts0tq0 | ts1tq1 | ts0tq1] (first two need the causal mask)
  - exp on ACT (one strided call per pair covering both subs); causal mask as
    one DVE multiply over the two triangle blocks (mtriT broadcast)
  - AV: lhsT = P_T block, rhs = v_aug -> O lands [tq, d] in PSUM and the
    softmax denominator appears for free in column 64 of each 65-wide block
  - normalize = DVE reciprocal [128,4] + ONE tensor_tensor multiply with the
    reciprocal broadcast along d (per-partition = per-tq -> cheap), which also
    serves as the PSUM->SBUF evacuation (bf16 cast)
  - per batch, O [tq, 384] is DMA-XBAR-transposed (2 calls) to OT [d, tq] for
    the output projection (32 transposes total vs 96 for per-pair P^T)
  - x loads and y stores ride the GpSimd SWDGE queue so the Sync queue only
    carries weights + O-transposes
  - group g+1's QK/V projection matmuls are interleaved as PE filler during
    group g's softmax waits; y-projections are deferred by ~1 pair so the
    PE never waits on the O-transpose chain
"""

import sys

sys.path.insert(0, "/opt/trn_rl_repo")

import numpy as np
import ml_dtypes

import concourse.bass as bass
import concourse.tile as tile
from concourse import mybir
from concourse.bass_utils import run_bass_kernel_spmd

def split_multi_waits(nc):
    """This walrus build accepts at most one sync-wait command per
    instruction; hoist extra waits into standalone InstEventSemaphore
    instructions on the same engine queue (queue waits run in order before
    the original instruction, so semantics are preserved)."""
    ctr = [0]

    def mk(engine, wait):
        ctr[0] += 1
        return mybir.InstEventSemaphore(
            name=f"WSPLIT-{ctr[0]}",
            engine=engine,
            ins=[],
            outs=[],
            sync_info=mybir.SyncInfo(on_wait=[wait], on_update=[]),
        )

    for f in nc.m.functions:
        for blk in f.blocks:
            insts = blk.instructions
            out = []
            for inst in insts:
                si = inst.sync_info
                if si is not None and len(si.on_wait) > 1:
                    waits = list(si.on_wait)
                    for w in waits[:-1]:
                        out.append(mk(inst.engine, w))
                    inst.sync_info = mybir.SyncInfo(
                        on_wait=[waits[-1]], on_update=list(si.on_update)
                    )
                out.append(inst)
            insts[:] = out
    return nc


N_CORES = 8
B, T, C = 128, 256, 384
H, DH = 6, 64
BL = B // N_CORES  # batches per core
GB = 2  # batches per projection group (N = GB*T = 512 <= one PSUM bank fp32)
NG = BL // GB
BF16 = mybir.dt.bfloat16
FP32 = mybir.dt.float32
AFT = mybir.ActivationFunctionType
SCALE = DH**-0.5  # 0.125


def build_kernel() -> bass.Bass:
    nc = bass.Bass()
    xT = nc.dram_tensor("xT", [BL, C, T], BF16, kind="ExternalInput")
    wqt = nc.dram_tensor("wqt", [C, C], BF16, kind="ExternalInput")  # Wq.T [C, D]
    wkt = nc.dram_tensor("wkt", [C, C], BF16, kind="ExternalInput")
    wvt = nc.dram_tensor("wvt", [C, C], BF16, kind="ExternalInput")
    wot = nc.dram_tensor("wot", [C, C], BF16, kind="ExternalInput")  # Wo.T [D, C]
    y = nc.dram_tensor("y", [BL, T, C], BF16, kind="ExternalOutput")

    with tile.TileContext(nc) as tc:
        with (
            tc.tile_pool(name="const", bufs=1) as const,
            tc.tile_pool(name="xp", bufs=2) as xp,
            tc.tile_pool(name="qkv", bufs=2) as qkv,
            tc.tile_pool(name="pp", bufs=3) as pp,
            tc.tile_pool(name="st", bufs=3) as st,
            tc.tile_pool(name="osb", bufs=3) as osb,
            tc.tile_pool(name="otp", bufs=3) as otp,
            tc.tile_pool(name="yp", bufs=3) as yp,
            tc.tile_pool(name="psProj", bufs=2, space="PSUM") as psProj,
            tc.tile_pool(name="psSc", bufs=2, space="PSUM") as psSc,
            tc.tile_pool(name="psPo", bufs=2, space="PSUM") as psPo,
        ):
            # prefetch x for group 0 (SWDGE queue) ahead of the (larger)
            # weight DMAs (sync queue) so the first projections start ASAP
            xt0 = xp.tile([128, 3, GB, T], BF16, name="xt_g0")
            for bi in range(GB):
                nc.gpsimd.dma_start(
                    out=xt0[:, :, bi, :],
                    in_=xT[bi].rearrange("(k p) t -> p k t", p=128),
                )
            # multiplicative causal mask for TRANSPOSED scores [ts, tq]:
            # keep tq >= ts, i.e. col >= partition (upper triangle + diag)
            mtriT = const.tile([128, 128], BF16)
            nc.gpsimd.memset(mtriT, 1.0)
            nc.gpsimd.affine_select(
                out=mtriT, in_=mtriT,
                compare_op=mybir.AluOpType.is_ge,
                fill=0.0, base=0, pattern=[[1, 128]], channel_multiplier=-1,
            )
            # tiny dummy exp: forces the ACT exp-table load during the DMA
            # wait instead of on the first real softmax
            dummy = const.tile([128, 2], FP32)
            nc.scalar.activation(dummy, mtriT[:, 0:2], AFT.Exp, scale=1.0)

            # weight loads spread over the three DMA-capable queues, ordered
            # by first use, so the first projections start ~1us in
            w_sb = {}
            for name, dram, eng in (
                ("wq", wqt, nc.sync),
                ("wk", wkt, nc.scalar),
                ("wv", wvt, nc.gpsimd),
                ("wo", wot, nc.sync),
            ):
                w = const.tile([128, 3, C], BF16, tag=name)
                eng.dma_start(out=w, in_=dram.rearrange("(k p) d -> p k d", p=128))
                w_sb[name] = w

            def load_group(g, xt=None):
                """DMA xT for group g, allocate qt/kt/v_aug tiles."""
                if xt is None:
                    xt = xp.tile([128, 3, GB, T], BF16, name=f"xt{g}")
                    for bi in range(GB):
                        nc.gpsimd.dma_start(
                            out=xt[:, :, bi, :],
                            in_=xT[g * GB + bi].rearrange(
                                "(k p) t -> p k t", p=128
                            ),
                        )
                qt = qkv.tile([128, 3, GB, T], BF16, tag="qt", name=f"qt{g}")
                kt = qkv.tile([128, 3, GB, T], BF16, tag="kt", name=f"kt{g}")
                vs = []
                for bi in range(GB):
                    # head stride 68 (not 65) keeps every rhs slice 8B-aligned
                    v = qkv.tile(
                        [128, 2, H, 68], BF16, tag=f"v{bi}", name=f"v{g}_{bi}"
                    )
                    # constant 1.0 column 64 -> AV matmul emits the softmax
                    # denominator for free
                    nc.gpsimd.memset(v[:, :, :, 64:65], 1.0)
                    vs.append(v)
                return xt, qt, kt, vs

            def proj_emitters(xt, qt, kt, vs):
                """Closures each emitting one PSUM-chunk of the QK/V
                projections (3 accumulating matmuls + 1 evacuation). Ordered
                so the consumers' dependencies resolve earliest-first."""
                def qk_em(dst, wname, d):
                    def em():
                        ps = psProj.tile([128, GB * T], FP32, tag="proj",
                                         name="psqk")
                        for k in range(3):
                            nc.tensor.matmul(
                                ps,
                                lhsT=w_sb[wname][:, k, d * 128:(d + 1) * 128],
                                rhs=xt[:, k, :, :],
                                start=(k == 0), stop=(k == 2),
                            )
                        nc.scalar.copy(dst[:, d, :, :], ps)
                    return em

                def v_em(bi, t2):
                    def em():
                        ps = psProj.tile([128, GB * T], FP32, tag="proj",
                                         name="psv")
                        for k in range(3):
                            nc.tensor.matmul(
                                ps[:, 0:C],
                                lhsT=xt[:, k, bi, t2 * 128:(t2 + 1) * 128],
                                rhs=w_sb["wv"][:, k, :],
                                start=(k == 0), stop=(k == 2),
                            )
                        nc.vector.tensor_copy(
                            vs[bi][:, t2, :, 0:64],
                            ps[:, 0:C].rearrange("p (h j) -> p h j", j=64),
                        )
                    return em

                return [
                    qk_em(qt, "wq", 0), qk_em(kt, "wk", 0),
                    v_em(0, 0), v_em(0, 1),
                    qk_em(qt, "wq", 1), qk_em(kt, "wk", 1),
                    qk_em(qt, "wq", 2), qk_em(kt, "wk", 2),
                    v_em(1, 0), v_em(1, 1),
                ]

            def att_stage1(qt, kt, bi, pair):
                """Scores (PE) + exp (ACT) + causal mask (DVE) -> masked P_T."""
                # ---- transposed scores S_T[ts, tq], fp32 psum ----
                # block order per sub: [ts0tq0 | ts1tq1 | ts0tq1]
                # (triangle blocks first so the mask is one contiguous slice)
                sc = psSc.tile([128, 2, 512], FP32, tag="sc", name="sc")
                for s in range(2):
                    doff = s * 64
                    kts = kt[doff:doff + 64, pair, bi, :]
                    qts = qt[doff:doff + 64, pair, bi, :]
                    nc.tensor.matmul(
                        sc[:, s, 0:128], lhsT=kts[:, 0:128],
                        rhs=qts[:, 0:128], start=True, stop=True,
                    )
                    nc.tensor.matmul(
                        sc[:, s, 256:384], lhsT=kts[:, 0:128],
                        rhs=qts[:, 128:256], start=True, stop=True,
                    )
                    nc.tensor.matmul(
                        sc[:, s, 128:256], lhsT=kts[:, 128:256],
                        rhs=qts[:, 128:256], start=True, stop=True,
                    )
                # ---- exp on ACT (both subs, one strided call) ----
                p_t = pp.tile([128, 2, 3, 128], BF16, tag="p")
                nc.scalar.activation(
                    p_t,
                    sc[:, :, 0:384].rearrange("p s (k c) -> p s k c", c=128),
                    AFT.Exp, scale=SCALE,
                )
                # ---- causal mask: one DVE multiply over the 2 triangle
                # blocks of both subs ----
                nc.vector.tensor_mul(
                    p_t[:, :, 0:2, :], p_t[:, :, 0:2, :],
                    mtriT[:, None, None, :].to_broadcast((128, 2, 2, 128)),
                )
                return p_t

            def att_stage2(p_t, v, pair, o_sbt):
                """AV matmuls + fused denominator + normalize/evacuate."""
                # block stride 66 fp32 = 264B keeps matmul PSUM outputs
                # 8B-aligned (PSUM cacheline)
                po = psPo.tile([128, 2, 2, 66], FP32, tag="po", name="po")
                for s in range(2):
                    h = 2 * pair + s
                    nc.tensor.matmul(
                        po[:, 0, s, 0:65], lhsT=p_t[:, s, 0, :],
                        rhs=v[:, 0, h, 0:65], start=True, stop=True,
                    )
                    nc.tensor.matmul(
                        po[:, 1, s, 0:65], lhsT=p_t[:, s, 2, :],
                        rhs=v[:, 0, h, 0:65], start=True, stop=False,
                    )
                    nc.tensor.matmul(
                        po[:, 1, s, 0:65], lhsT=p_t[:, s, 1, :],
                        rhs=v[:, 1, h, 0:65], start=False, stop=True,
                    )
                # ---- normalize: per-partition (=per-tq) reciprocal, then
                # one broadcast multiply that doubles as the PSUM->SBUF
                # evacuation ----
                rs = st.tile([128, 2, 2], FP32, tag="rs")
                nc.vector.reciprocal(rs, po[:, :, :, 64])
                out_sl = o_sbt[:, :, pair * 128:(pair + 1) * 128].rearrange(
                    "p t (s j) -> p t s j", j=64
                )
                nc.vector.tensor_mul(
                    out_sl, po[:, :, :, 0:64],
                    rs[:, :, :, None].to_broadcast((128, 2, 2, 64)),
                )

            def emit_trans(o_sbt):
                otp_t = otp.tile([128, 2, 3, 128], BF16)
                for tqb in range(2):
                    nc.sync.dma_start(
                        out=otp_t[:, tqb, :, :],
                        in_=o_sbt[:, tqb, :].rearrange("p (k c) -> p k c", c=128),
                        transpose=True,
                    )
                return otp_t

            def emit_yproj(b, otp_t):
                ys = yp.tile([128, 2, C], BF16)
                for tqb in range(2):
                    ps = psProj.tile([128, GB * T], FP32, tag="proj", name="psy")
                    for k in range(3):
                        nc.tensor.matmul(
                            ps[:, 0:C],
                            lhsT=otp_t[:, tqb, k, :],
                            rhs=w_sb["wo"][:, k, :],
                            start=(k == 0), stop=(k == 2),
                        )
                    nc.vector.tensor_copy(ys[:, tqb, :], ps[:, 0:C])
                nc.gpsimd.dma_start(
                    out=y[b].rearrange("(t2 p) c -> p t2 c", p=128), in_=ys
                )

            # ---- prologue: only the chunks pair (b0, p0) needs up front;
            # the rest of group 0's projections become its own filler ----
            cur = load_group(0, xt=xt0)
            g0_ems = proj_emitters(cur[0], cur[1], cur[2], cur[3])
            for em in g0_ems[:4]:
                em()
            carry = g0_ems[4:]
            pending_y = []
            pending2 = [None]  # one-deep attention software pipeline

            def flush_y():
                while pending_y:
                    b, otp_t = pending_y.pop(0)
                    emit_yproj(b, otp_t)

            def run_pending2():
                if pending2[0] is not None:
                    pending2[0]()
                    pending2[0] = None

            for g in range(NG):
                nxt = None
                nxt_ems = list(carry)
                carry = []
                if g + 1 < NG:
                    nxt = load_group(g + 1)
                    nxt_ems += proj_emitters(nxt[0], nxt[1], nxt[2], nxt[3])
                _, qt, kt, vs = cur
                ei = [0]

                def filler(nxt_ems=nxt_ems, ei=ei):
                    # one projection chunk, emitted inside the softmax wait
                    # so the PE always has independent work
                    if ei[0] < len(nxt_ems):
                        nxt_ems[ei[0]]()
                        ei[0] += 1

                for bi in range(GB):
                    b = g * GB + bi
                    o_sbt = osb.tile([128, 2, C], BF16)
                    for pair in range(3):
                        filler()
                        p_t = att_stage1(qt, kt, bi, pair)
                        # previous att's AV+norm lands here, giving the PE
                        # ready work while this pair's exp/mask run
                        run_pending2()
                        filler()
                        if pair == 2:
                            flush_y()

                        def stage2(p_t=p_t, v=vs[bi], pair=pair,
                                   o_sbt=o_sbt, b=b):
                            att_stage2(p_t, v, pair, o_sbt)
                            if pair == 2:
                                pending_y.append((b, emit_trans(o_sbt)))
                        pending2[0] = stage2
                while ei[0] < len(nxt_ems):
                    nxt_ems[ei[0]]()
                    ei[0] += 1
                cur = nxt
            run_pending2()
            flush_y()
    return nc


_NC = None


def _get_nc():
    global _NC
    if _NC is None:
        _NC = split_multi_waits(build_kernel())
    return _NC


def kernel(x, Wq, Wk, Wv, Wo, _trace=False):
    bf16 = ml_dtypes.bfloat16
    wq_t = np.ascontiguousarray(Wq.T).astype(bf16)
    wk_t = np.ascontiguousarray(Wk.T).astype(bf16)
    wv_t = np.ascontiguousarray(Wv.T).astype(bf16)
    wo_t = np.ascontiguousarray(Wo.T).astype(bf16)
    in_maps = []
    for i in range(N_CORES):
        xs = x[i * BL : (i + 1) * BL]  # [BL, T, C]
        xs_t = np.ascontiguousarray(xs.transpose(0, 2, 1)).astype(bf16)
        in_maps.append(
            {"xT": xs_t, "wqt": wq_t, "wkt": wk_t, "wvt": wv_t, "wot": wo_t}
        )
    res = run_bass_kernel_spmd(
        _get_nc(), in_maps, list(range(N_CORES)), trace=_trace
    )
    out = np.concatenate([r["y"] for r in res.results], axis=0)
    if _trace:
        return out.astype(np.float32), res
    return out.astype(np.float32)


# revision 12
# speedup vs baseline: 1.8584x; 1.0626x over previous
"""Causal multi-head attention (B=128, T=256, C=384, H=6, Dh=64) on 8 TRN2
NeuronCores, data-parallel over batch (16 batches per core, no collectives).

Layout strategy per core (v5 — transposed scores, fused denominator):
  - host pre-transposes x to xT [b, C, T] and casts activations/weights to bf16
  - QT/KT computed as [D, T] (Dh on partitions); V computed as [T, H, 65]
    with a constant-1.0 65th column per head ("v_aug")
  - scores are computed TRANSPOSED: S_T[ts, tq] = K_blk.T-style matmuls with
    kt as the stationary operand, in three 128x128 blocks per sub-head
    ordered [ts0tq0 | ts1tq1 | ts0tq1] (first two need the causal mask)
  - exp on ACT (one strided call per pair covering both subs); causal mask as
    one DVE multiply over the two triangle blocks (mtriT broadcast)
  - AV: lhsT = P_T block, rhs = v_aug -> O lands [tq, d] in PSUM and the
    softmax denominator appears for free in column 64 of each 65-wide block
  - normalize = DVE reciprocal [128,4] + ONE tensor_tensor multiply with the
    reciprocal broadcast along d (per-partition = per-tq -> cheap), which also
    serves as the PSUM->SBUF evacuation (bf16 cast)
  - per batch, O [tq, 384] is DMA-XBAR-transposed (2 calls) to OT [d, tq] for
    the output projection (32 transposes total vs 96 for per-pair P^T)
  - x loads and y stores ride the GpSimd SWDGE queue so the Sync queue only
    carries weights + O-transposes
  - group g+1's QK/V projection matmuls are interleaved as PE filler during
    group g's softmax waits; y-projections are deferred by ~1 pair so the
    PE never waits on the O-transpose chain
"""

import sys

sys.path.insert(0, "/opt/trn_rl_repo")

import numpy as np
import ml_dtypes

import concourse.bass as bass
import concourse.tile as tile
from concourse import mybir
from concourse.bass_utils import run_bass_kernel_spmd

def split_multi_waits(nc):
    """This walrus build accepts at most one sync-wait command per
    instruction; hoist extra waits into standalone InstEventSemaphore
    instructions on the same engine queue (queue waits run in order before
    the original instruction, so semantics are preserved)."""
    ctr = [0]

    def mk(engine, wait):
        ctr[0] += 1
        return mybir.InstEventSemaphore(
            name=f"WSPLIT-{ctr[0]}",
            engine=engine,
            ins=[],
            outs=[],
            sync_info=mybir.SyncInfo(on_wait=[wait], on_update=[]),
        )

    for f in nc.m.functions:
        for blk in f.blocks:
            insts = blk.instructions
            out = []
            for inst in insts:
                si = inst.sync_info
                if si is not None and len(si.on_wait) > 1:
                    waits = list(si.on_wait)
                    for w in waits[:-1]:
                        out.append(mk(inst.engine, w))
                    inst.sync_info = mybir.SyncInfo(
                        on_wait=[waits[-1]], on_update=list(si.on_update)
                    )
                out.append(inst)
            insts[:] = out
    return nc


N_CORES = 8
B, T, C = 128, 256, 384
H, DH = 6, 64
BL = B // N_CORES  # batches per core
GB = 2  # batches per projection group (N = GB*T = 512 <= one PSUM bank fp32)
NG = BL // GB
BF16 = mybir.dt.bfloat16
FP32 = mybir.dt.float32
AFT = mybir.ActivationFunctionType
SCALE = DH**-0.5  # 0.125


def build_kernel() -> bass.Bass:
    nc = bass.Bass()
    xT = nc.dram_tensor("xT", [BL, C, T], BF16, kind="ExternalInput")
    wqt = nc.dram_tensor("wqt", [C, C], BF16, kind="ExternalInput")  # Wq.T [C, D]
    wkt = nc.dram_tensor("wkt", [C, C], BF16, kind="ExternalInput")
    wvt = nc.dram_tensor("wvt", [C, C], BF16, kind="ExternalInput")
    wot = nc.dram_tensor("wot", [C, C], BF16, kind="ExternalInput")  # Wo.T [D, C]
    y = nc.dram_tensor("y", [BL, T, C], BF16, kind="ExternalOutput")

    with tile.TileContext(nc) as tc:
        with (
            tc.tile_pool(name="const", bufs=1) as const,
            tc.tile_pool(name="xp", bufs=2) as xp,
            tc.tile_pool(name="qkv", bufs=2) as qkv,
            tc.tile_pool(name="pp", bufs=4) as pp,
            tc.tile_pool(name="st", bufs=4) as st,
            tc.tile_pool(name="osb", bufs=3) as osb,
            tc.tile_pool(name="otp", bufs=3) as otp,
            tc.tile_pool(name="yp", bufs=3) as yp,
            tc.tile_pool(name="psProj", bufs=2, space="PSUM") as psProj,
            tc.tile_pool(name="psSc", bufs=2, space="PSUM") as psSc,
            tc.tile_pool(name="psPo", bufs=2, space="PSUM") as psPo,
        ):
            # prefetch x for group 0 (SWDGE queue) ahead of the (larger)
            # weight DMAs (sync queue) so the first projections start ASAP
            xt0 = xp.tile([128, 3, GB, T], BF16, name="xt_g0")
            for bi in range(GB):
                nc.gpsimd.dma_start(
                    out=xt0[:, :, bi, :],
                    in_=xT[bi].rearrange("(k p) t -> p k t", p=128),
                )
            # multiplicative causal mask for TRANSPOSED scores [ts, tq]:
            # keep tq >= ts, i.e. col >= partition (upper triangle + diag)
            mtriT = const.tile([128, 128], BF16)
            nc.gpsimd.memset(mtriT, 1.0)
            nc.gpsimd.affine_select(
                out=mtriT, in_=mtriT,
                compare_op=mybir.AluOpType.is_ge,
                fill=0.0, base=0, pattern=[[1, 128]], channel_multiplier=-1,
            )
            # tiny dummy exp: forces the ACT exp-table load during the DMA
            # wait instead of on the first real softmax
            dummy = const.tile([128, 2], FP32)
            nc.scalar.activation(dummy, mtriT[:, 0:2], AFT.Exp, scale=1.0)

            # weight loads spread over the three DMA-capable queues, ordered
            # by first use, so the first projections start ~1us in
            w_sb = {}
            for name, dram, eng in (
                ("wq", wqt, nc.sync),
                ("wk", wkt, nc.scalar),
                ("wv", wvt, nc.gpsimd),
                ("wo", wot, nc.sync),
            ):
                w = const.tile([128, 3, C], BF16, tag=name)
                eng.dma_start(out=w, in_=dram.rearrange("(k p) d -> p k d", p=128))
                w_sb[name] = w

            def load_group(g, xt=None):
                """DMA xT for group g, allocate qt/kt/v_aug tiles."""
                if xt is None:
                    xt = xp.tile([128, 3, GB, T], BF16, name=f"xt{g}")
                    for bi in range(GB):
                        nc.gpsimd.dma_start(
                            out=xt[:, :, bi, :],
                            in_=xT[g * GB + bi].rearrange(
                                "(k p) t -> p k t", p=128
                            ),
                        )
                qt = qkv.tile([128, 3, GB, T], BF16, tag="qt", name=f"qt{g}")
                kt = qkv.tile([128, 3, GB, T], BF16, tag="kt", name=f"kt{g}")
                vs = []
                for bi in range(GB):
                    # head stride 68 (not 65) keeps every rhs slice 8B-aligned
                    v = qkv.tile(
                        [128, 2, H, 68], BF16, tag=f"v{bi}", name=f"v{g}_{bi}"
                    )
                    # constant 1.0 column 64 -> AV matmul emits the softmax
                    # denominator for free
                    nc.gpsimd.memset(v[:, :, :, 64:65], 1.0)
                    vs.append(v)
                return xt, qt, kt, vs

            def proj_emitters(xt, qt, kt, vs):
                """Closures each emitting one PSUM-chunk of the QK/V
                projections (3 accumulating matmuls + 1 evacuation). Ordered
                so the consumers' dependencies resolve earliest-first."""
                def qk_em(dst, wname, d):
                    def em():
                        ps = psProj.tile([128, GB * T], FP32, tag="proj",
                                         name="psqk")
                        for k in range(3):
                            nc.tensor.matmul(
                                ps,
                                lhsT=w_sb[wname][:, k, d * 128:(d + 1) * 128],
                                rhs=xt[:, k, :, :],
                                start=(k == 0), stop=(k == 2),
                            )
                        nc.scalar.copy(dst[:, d, :, :], ps)
                    return em

                def v_em(bi, t2):
                    def em():
                        ps = psProj.tile([128, GB * T], FP32, tag="proj",
                                         name="psv")
                        for k in range(3):
                            nc.tensor.matmul(
                                ps[:, 0:C],
                                lhsT=xt[:, k, bi, t2 * 128:(t2 + 1) * 128],
                                rhs=w_sb["wv"][:, k, :],
                                start=(k == 0), stop=(k == 2),
                            )
                        nc.vector.tensor_copy(
                            vs[bi][:, t2, :, 0:64],
                            ps[:, 0:C].rearrange("p (h j) -> p h j", j=64),
                        )
                    return em

                return [
                    qk_em(qt, "wq", 0), qk_em(kt, "wk", 0),
                    v_em(0, 0), v_em(0, 1),
                    qk_em(qt, "wq", 1), qk_em(kt, "wk", 1),
                    qk_em(qt, "wq", 2), qk_em(kt, "wk", 2),
                    v_em(1, 0), v_em(1, 1),
                ]

            def att_stage1(qt, kt, bi, pair):
                """Scores (PE) + exp (ACT) + causal mask (DVE) -> masked P_T."""
                # ---- transposed scores S_T[ts, tq], fp32 psum ----
                # block order per sub: [ts0tq0 | ts1tq1 | ts0tq1]
                # (triangle blocks first so the mask is one contiguous slice)
                sc = psSc.tile([128, 2, 512], FP32, tag="sc", name="sc")
                for s in range(2):
                    doff = s * 64
                    kts = kt[doff:doff + 64, pair, bi, :]
                    qts = qt[doff:doff + 64, pair, bi, :]
                    nc.tensor.matmul(
                        sc[:, s, 0:128], lhsT=kts[:, 0:128],
                        rhs=qts[:, 0:128], start=True, stop=True,
                    )
                    nc.tensor.matmul(
                        sc[:, s, 256:384], lhsT=kts[:, 0:128],
                        rhs=qts[:, 128:256], start=True, stop=True,
                    )
                    nc.tensor.matmul(
                        sc[:, s, 128:256], lhsT=kts[:, 128:256],
                        rhs=qts[:, 128:256], start=True, stop=True,
                    )
                # ---- exp on ACT (both subs, one strided call) ----
                p_t = pp.tile([128, 2, 3, 128], BF16, tag="p")
                nc.scalar.activation(
                    p_t,
                    sc[:, :, 0:384].rearrange("p s (k c) -> p s k c", c=128),
                    AFT.Exp, scale=SCALE,
                )
                # ---- causal mask: one DVE multiply over the 2 triangle
                # blocks of both subs ----
                nc.vector.tensor_mul(
                    p_t[:, :, 0:2, :], p_t[:, :, 0:2, :],
                    mtriT[:, None, None, :].to_broadcast((128, 2, 2, 128)),
                )
                return p_t

            def att_stage2(p_t, v, pair, o_sbt):
                """AV matmuls + fused denominator + normalize/evacuate."""
                # block stride 66 fp32 = 264B keeps matmul PSUM outputs
                # 8B-aligned (PSUM cacheline)
                po = psPo.tile([128, 2, 2, 66], FP32, tag="po", name="po")
                for s in range(2):
                    h = 2 * pair + s
                    nc.tensor.matmul(
                        po[:, 0, s, 0:65], lhsT=p_t[:, s, 0, :],
                        rhs=v[:, 0, h, 0:65], start=True, stop=True,
                    )
                    nc.tensor.matmul(
                        po[:, 1, s, 0:65], lhsT=p_t[:, s, 2, :],
                        rhs=v[:, 0, h, 0:65], start=True, stop=False,
                    )
                    nc.tensor.matmul(
                        po[:, 1, s, 0:65], lhsT=p_t[:, s, 1, :],
                        rhs=v[:, 1, h, 0:65], start=False, stop=True,
                    )
                # ---- normalize: per-partition (=per-tq) reciprocal, then
                # one broadcast multiply that doubles as the PSUM->SBUF
                # evacuation ----
                rs = st.tile([128, 2, 2], FP32, tag="rs")
                nc.vector.reciprocal(rs, po[:, :, :, 64])
                out_sl = o_sbt[:, :, pair * 128:(pair + 1) * 128].rearrange(
                    "p t (s j) -> p t s j", j=64
                )
                nc.vector.tensor_mul(
                    out_sl, po[:, :, :, 0:64],
                    rs[:, :, :, None].to_broadcast((128, 2, 2, 64)),
                )

            def emit_trans(o_sbt):
                otp_t = otp.tile([128, 2, 3, 128], BF16)
                for tqb in range(2):
                    nc.sync.dma_start(
                        out=otp_t[:, tqb, :, :],
                        in_=o_sbt[:, tqb, :].rearrange("p (k c) -> p k c", c=128),
                        transpose=True,
                    )
                return otp_t

            def emit_yproj(b, otp_t):
                ys = yp.tile([128, 2, C], BF16)
                for tqb in range(2):
                    ps = psProj.tile([128, GB * T], FP32, tag="proj", name="psy")
                    for k in range(3):
                        nc.tensor.matmul(
                            ps[:, 0:C],
                            lhsT=otp_t[:, tqb, k, :],
                            rhs=w_sb["wo"][:, k, :],
                            start=(k == 0), stop=(k == 2),
                        )
                    nc.vector.tensor_copy(ys[:, tqb, :], ps[:, 0:C])
                nc.gpsimd.dma_start(
                    out=y[b].rearrange("(t2 p) c -> p t2 c", p=128), in_=ys
                )

            # ---- prologue: only the chunks pair (b0, p0) needs up front;
            # the rest of group 0's projections become its own filler ----
            cur = load_group(0, xt=xt0)
            g0_ems = proj_emitters(cur[0], cur[1], cur[2], cur[3])
            for em in g0_ems[:4]:
                em()
            carry = g0_ems[4:]
            pending_y = []
            pending2 = []  # two-deep attention software pipeline (AV+norm
            # of att k runs after scores/exp/mask of att k+2 are emitted)

            def flush_y():
                while pending_y:
                    b, otp_t = pending_y.pop(0)
                    emit_yproj(b, otp_t)

            def run_pending2(keep=0):
                while len(pending2) > keep:
                    pending2.pop(0)()

            for g in range(NG):
                nxt = None
                nxt_ems = list(carry)
                carry = []
                if g + 1 < NG:
                    nxt = load_group(g + 1)
                    nxt_ems += proj_emitters(nxt[0], nxt[1], nxt[2], nxt[3])
                _, qt, kt, vs = cur
                ei = [0]

                def filler(nxt_ems=nxt_ems, ei=ei):
                    # one projection chunk, emitted inside the softmax wait
                    # so the PE always has independent work
                    if ei[0] < len(nxt_ems):
                        nxt_ems[ei[0]]()
                        ei[0] += 1

                for bi in range(GB):
                    b = g * GB + bi
                    o_sbt = osb.tile([128, 2, C], BF16)
                    for pair in range(3):
                        filler()
                        p_t = att_stage1(qt, kt, bi, pair)
                        # the AV+norm of the att two pairs back lands here,
                        # giving the PE ready work while exp/mask run
                        run_pending2(keep=1)
                        filler()
                        if pair == 2:
                            flush_y()

                        def stage2(p_t=p_t, v=vs[bi], pair=pair,
                                   o_sbt=o_sbt, b=b):
                            att_stage2(p_t, v, pair, o_sbt)
                            if pair == 2:
                                pending_y.append((b, emit_trans(o_sbt)))
                        pending2.append(stage2)
                while ei[0] < len(nxt_ems):
                    nxt_ems[ei[0]]()
                    ei[0] += 1
                cur = nxt
            run_pending2(keep=0)
            flush_y()
    return nc


_NC = None


def _get_nc():
    global _NC
    if _NC is None:
        _NC = split_multi_waits(build_kernel())
    return _NC


def kernel(x, Wq, Wk, Wv, Wo, _trace=False):
    bf16 = ml_dtypes.bfloat16
    wq_t = np.ascontiguousarray(Wq.T).astype(bf16)
    wk_t = np.ascontiguousarray(Wk.T).astype(bf16)
    wv_t = np.ascontiguousarray(Wv.T).astype(bf16)
    wo_t = np.ascontiguousarray(Wo.T).astype(bf16)
    in_maps = []
    for i in range(N_CORES):
        xs = x[i * BL : (i + 1) * BL]  # [BL, T, C]
        xs_t = np.ascontiguousarray(xs.transpose(0, 2, 1)).astype(bf16)
        in_maps.append(
            {"xT": xs_t, "wqt": wq_t, "wkt": wk_t, "wvt": wv_t, "wot": wo_t}
        )
    res = run_bass_kernel_spmd(
        _get_nc(), in_maps, list(range(N_CORES)), trace=_trace
    )
    out = np.concatenate([r["y"] for r in res.results], axis=0)
    if _trace:
        return out.astype(np.float32), res
    return out.astype(np.float32)


# revision 18
# speedup vs baseline: 1.9513x; 1.0500x over previous
"""Causal multi-head attention (B=128, T=256, C=384, H=6, Dh=64) on 8 TRN2
NeuronCores, data-parallel over batch (16 batches per core, no collectives).

Layout strategy per core (v5 — transposed scores, fused denominator):
  - host pre-transposes x to xT [b, C, T] and casts activations/weights to bf16
  - QT/KT computed as [D, T] (Dh on partitions); V computed as [T, H, 65]
    with a constant-1.0 65th column per head ("v_aug")
  - scores are computed TRANSPOSED: S_T[ts, tq] = K_blk.T-style matmuls with
    kt as the stationary operand, in three 128x128 blocks per sub-head
    ordered [ts0tq0 | ts1tq1 | ts0tq1] (first two need the causal mask)
  - exp on ACT (one strided call per pair covering both subs); causal mask as
    one DVE multiply over the two triangle blocks (mtriT broadcast)
  - AV: lhsT = P_T block, rhs = v_aug -> O lands [tq, d] in PSUM and the
    softmax denominator appears for free in column 64 of each 65-wide block
  - normalize = DVE reciprocal [128,4] + ONE tensor_tensor multiply with the
    reciprocal broadcast along d (per-partition = per-tq -> cheap), which also
    serves as the PSUM->SBUF evacuation (bf16 cast)
  - per batch, O [tq, 384] is DMA-XBAR-transposed (2 calls) to OT [d, tq] for
    the output projection (32 transposes total vs 96 for per-pair P^T)
  - x loads and y stores ride the GpSimd SWDGE queue so the Sync queue only
    carries weights + O-transposes
  - group g+1's QK/V projection matmuls are interleaved as PE filler during
    group g's softmax waits; y-projections are deferred by ~1 pair so the
    PE never waits on the O-transpose chain
"""

import sys

sys.path.insert(0, "/opt/trn_rl_repo")

import numpy as np
import ml_dtypes

import concourse.bass as bass
import concourse.tile as tile
from concourse import mybir
from concourse.bass_utils import run_bass_kernel_spmd

def split_multi_waits(nc):
    """This walrus build accepts at most one sync-wait command per
    instruction; hoist extra waits into standalone InstEventSemaphore
    instructions on the same engine queue (queue waits run in order before
    the original instruction, so semantics are preserved)."""
    ctr = [0]

    def mk(engine, wait):
        ctr[0] += 1
        return mybir.InstEventSemaphore(
            name=f"WSPLIT-{ctr[0]}",
            engine=engine,
            ins=[],
            outs=[],
            sync_info=mybir.SyncInfo(on_wait=[wait], on_update=[]),
        )

    for f in nc.m.functions:
        for blk in f.blocks:
            insts = blk.instructions
            out = []
            for inst in insts:
                si = inst.sync_info
                if si is not None and len(si.on_wait) > 1:
                    waits = list(si.on_wait)
                    for w in waits[:-1]:
                        out.append(mk(inst.engine, w))
                    inst.sync_info = mybir.SyncInfo(
                        on_wait=[waits[-1]], on_update=list(si.on_update)
                    )
                out.append(inst)
            insts[:] = out
    return nc


N_CORES = 8
B, T, C = 128, 256, 384
H, DH = 6, 64
BL = B // N_CORES  # batches per core
GB = 2  # batches per projection group (N = GB*T = 512 <= one PSUM bank fp32)
NG = BL // GB
BF16 = mybir.dt.bfloat16
FP32 = mybir.dt.float32
AFT = mybir.ActivationFunctionType
SCALE = DH**-0.5  # 0.125


def build_kernel() -> bass.Bass:
    nc = bass.Bass()
    xT = nc.dram_tensor("xT", [BL, C, T], BF16, kind="ExternalInput")
    wqt = nc.dram_tensor("wqt", [C, C], BF16, kind="ExternalInput")  # Wq.T [C, D]
    wkt = nc.dram_tensor("wkt", [C, C], BF16, kind="ExternalInput")
    wvt = nc.dram_tensor("wvt", [C, C], BF16, kind="ExternalInput")
    wot = nc.dram_tensor("wot", [C, C], BF16, kind="ExternalInput")  # Wo.T [D, C]
    y = nc.dram_tensor("y", [BL, T, C], BF16, kind="ExternalOutput")

    with tile.TileContext(nc) as tc:
        with (
            tc.tile_pool(name="const", bufs=1) as const,
            tc.tile_pool(name="xp", bufs=2) as xp,
            tc.tile_pool(name="qkv", bufs=2) as qkv,
            tc.tile_pool(name="pp", bufs=4) as pp,
            tc.tile_pool(name="st", bufs=4) as st,
            tc.tile_pool(name="osb", bufs=3) as osb,
            tc.tile_pool(name="otp", bufs=3) as otp,
            tc.tile_pool(name="yp", bufs=3) as yp,
            tc.tile_pool(name="psProj", bufs=2, space="PSUM") as psProj,
            tc.tile_pool(name="psSc", bufs=2, space="PSUM") as psSc,
            tc.tile_pool(name="psPo", bufs=2, space="PSUM") as psPo,
        ):
            # prefetch x for group 0 (SWDGE queue) ahead of the (larger)
            # weight DMAs (sync queue) so the first projections start ASAP
            xt0 = xp.tile([128, 3, GB, T], BF16, name="xt_g0")
            for bi in range(GB):
                nc.gpsimd.dma_start(
                    out=xt0[:, :, bi, :],
                    in_=xT[bi].rearrange("(k p) t -> p k t", p=128),
                )
            # tiny dummy exp: forces the ACT exp-table load during the DMA
            # wait instead of on the first real softmax
            seed = const.tile([128, 2], BF16)
            nc.gpsimd.memset(seed, 1.0)
            dummy = const.tile([128, 2], FP32)
            nc.scalar.activation(dummy, seed, AFT.Exp, scale=1.0)

            # weight loads spread over the three DMA-capable queues, ordered
            # by first use, so the first projections start ~1us in
            w_sb = {}
            for name, dram, eng in (
                ("wq", wqt, nc.sync),
                ("wk", wkt, nc.scalar),
                ("wv", wvt, nc.gpsimd),
                ("wo", wot, nc.sync),
            ):
                w = const.tile([128, 3, C], BF16, tag=name)
                eng.dma_start(out=w, in_=dram.rearrange("(k p) d -> p k d", p=128))
                w_sb[name] = w

            def load_group(g, xt=None):
                """DMA xT for group g, allocate qt/kt/v_aug tiles."""
                if xt is None:
                    xt = xp.tile([128, 3, GB, T], BF16, name=f"xt{g}")
                    for bi in range(GB):
                        nc.gpsimd.dma_start(
                            out=xt[:, :, bi, :],
                            in_=xT[g * GB + bi].rearrange(
                                "(k p) t -> p k t", p=128
                            ),
                        )
                qt = qkv.tile([128, 3, GB, T], BF16, tag="qt", name=f"qt{g}")
                kt = qkv.tile([128, 3, GB, T], BF16, tag="kt", name=f"kt{g}")
                vs = []
                for bi in range(GB):
                    # head stride 68 (not 65) keeps every rhs slice 8B-aligned
                    v = qkv.tile(
                        [128, 2, H, 68], BF16, tag=f"v{bi}", name=f"v{g}_{bi}"
                    )
                    # constant 1.0 column 64 -> AV matmul emits the softmax
                    # denominator for free
                    nc.gpsimd.memset(v[:, :, :, 64:65], 1.0)
                    vs.append(v)
                return xt, qt, kt, vs

            def proj_emitters(xt, qt, kt, vs):
                """Closures each emitting one PSUM-chunk of the QK/V
                projections (3 accumulating matmuls + 1 evacuation). Ordered
                so the consumers' dependencies resolve earliest-first."""
                def qk_em(dst, wname, d):
                    def em():
                        ps = psProj.tile([128, GB * T], FP32, tag="proj",
                                         name="psqk")
                        for k in range(3):
                            nc.tensor.matmul(
                                ps,
                                lhsT=w_sb[wname][:, k, d * 128:(d + 1) * 128],
                                rhs=xt[:, k, :, :],
                                start=(k == 0), stop=(k == 2),
                            )
                        # chunk 1 evacuates on DVE to balance ACT/DVE load
                        if d == 1:
                            nc.vector.tensor_copy(dst[:, d, :, :], ps)
                        else:
                            nc.scalar.copy(dst[:, d, :, :], ps)
                    return em

                def v_em(bi, t2):
                    def em():
                        ps = psProj.tile([128, GB * T], FP32, tag="proj",
                                         name="psv")
                        for k in range(3):
                            nc.tensor.matmul(
                                ps[:, 0:C],
                                lhsT=xt[:, k, bi, t2 * 128:(t2 + 1) * 128],
                                rhs=w_sb["wv"][:, k, :],
                                start=(k == 0), stop=(k == 2),
                            )
                        nc.vector.tensor_copy(
                            vs[bi][:, t2, :, 0:64],
                            ps[:, 0:C].rearrange("p (h j) -> p h j", j=64),
                        )
                    return em

                return [
                    qk_em(qt, "wq", 0), qk_em(kt, "wk", 0),
                    v_em(0, 0), v_em(0, 1),
                    qk_em(qt, "wq", 1), qk_em(kt, "wk", 1),
                    qk_em(qt, "wq", 2), qk_em(kt, "wk", 2),
                    v_em(1, 0), v_em(1, 1),
                ]

            def att_stage1(qt, kt, bi, pair):
                """Scores (PE) + exp (ACT) + causal mask (DVE) -> masked P_T."""
                # ---- transposed scores S_T[ts, tq], fp32 psum ----
                # block order per sub: [ts0tq0 | ts1tq1 | ts0tq1]
                # (triangle blocks first so the mask is one contiguous slice)
                sc = psSc.tile([128, 2, 512], FP32, tag="sc", name="sc")
                kts = [kt[s * 64:s * 64 + 64, pair, bi, :] for s in range(2)]
                qts = [qt[s * 64:s * 64 + 64, pair, bi, :] for s in range(2)]
                # emission alternates subs so consecutive matmuls sit in
                # different PE row-groups (rows 0-63 vs 64-127): the 64-deep
                # reorder window pulls the other sub's LDWEIGHTS ahead and the
                # 32x32-tiled array can overlap their execution
                # (ts block, tq block, dst col) per layout [ts0tq0|ts1tq1|ts0tq1]
                for lo, ro, dst in ((0, 0, 0), (0, 128, 256), (128, 128, 128)):
                    for s in range(2):
                        nc.tensor.matmul(
                            sc[:, s, dst:dst + 128],
                            lhsT=kts[s][:, lo:lo + 128],
                            rhs=qts[s][:, ro:ro + 128],
                            start=True, stop=True,
                        )
                # ---- exp on ACT (both subs, one strided call) ----
                p_t = pp.tile([128, 2, 3, 128], BF16, tag="p")
                nc.scalar.activation(
                    p_t,
                    sc[:, :, 0:384].rearrange("p s (k c) -> p s k c", c=128),
                    AFT.Exp, scale=SCALE,
                )
                # ---- causal mask on GpSimd (otherwise idle): zero-step
                # pattern dims repeat the keep-(col >= partition) triangle
                # over all 4 (sub, block) slices in one call ----
                nc.gpsimd.affine_select(
                    out=p_t[:, :, 0:2, :], in_=p_t[:, :, 0:2, :],
                    compare_op=mybir.AluOpType.is_ge,
                    fill=0.0, base=0,
                    pattern=[[0, 2], [0, 2], [1, 128]],
                    channel_multiplier=-1,
                )
                return p_t

            def att_stage2(p_t, v, pair, o_sbt):
                """AV matmuls + fused denominator + normalize/evacuate."""
                # block stride 66 fp32 = 264B keeps matmul PSUM outputs
                # 8B-aligned (PSUM cacheline)
                po = psPo.tile([128, 2, 2, 66], FP32, tag="po", name="po")
                for s in range(2):
                    h = 2 * pair + s
                    nc.tensor.matmul(
                        po[:, 0, s, 0:65], lhsT=p_t[:, s, 0, :],
                        rhs=v[:, 0, h, 0:65], start=True, stop=True,
                    )
                    nc.tensor.matmul(
                        po[:, 1, s, 0:65], lhsT=p_t[:, s, 2, :],
                        rhs=v[:, 0, h, 0:65], start=True, stop=False,
                    )
                    nc.tensor.matmul(
                        po[:, 1, s, 0:65], lhsT=p_t[:, s, 1, :],
                        rhs=v[:, 1, h, 0:65], start=False, stop=True,
                    )
                # ---- normalize: per-partition (=per-tq) reciprocal, then
                # one broadcast multiply that doubles as the PSUM->SBUF
                # evacuation ----
                rs = st.tile([128, 2, 2], FP32, tag="rs")
                nc.vector.reciprocal(rs, po[:, :, :, 64])
                out_sl = o_sbt[:, :, pair * 128:(pair + 1) * 128].rearrange(
                    "p t (s j) -> p t s j", j=64
                )
                nc.vector.tensor_mul(
                    out_sl, po[:, :, :, 0:64],
                    rs[:, :, :, None].to_broadcast((128, 2, 2, 64)),
                )

            def emit_trans(o_sbt):
                otp_t = otp.tile([128, 2, 3, 128], BF16)
                for tqb in range(2):
                    nc.sync.dma_start(
                        out=otp_t[:, tqb, :, :],
                        in_=o_sbt[:, tqb, :].rearrange("p (k c) -> p k c", c=128),
                        transpose=True,
                    )
                return otp_t

            def emit_yproj(b, otp_t):
                ys = yp.tile([128, 2, C], BF16)
                for tqb in range(2):
                    ps = psProj.tile([128, GB * T], FP32, tag="proj", name="psy")
                    for k in range(3):
                        nc.tensor.matmul(
                            ps[:, 0:C],
                            lhsT=otp_t[:, tqb, k, :],
                            rhs=w_sb["wo"][:, k, :],
                            start=(k == 0), stop=(k == 2),
                        )
                    nc.vector.tensor_copy(ys[:, tqb, :], ps[:, 0:C])
                # y stores on the sync HWDGE queue: keeps the end-of-kernel
                # SWDGE drain off the critical tail
                nc.sync.dma_start(
                    out=y[b].rearrange("(t2 p) c -> p t2 c", p=128), in_=ys
                )

            # ---- prologue: only the chunks pair (b0, p0) needs up front;
            # the rest of group 0's projections become its own filler ----
            cur = load_group(0, xt=xt0)
            g0_ems = proj_emitters(cur[0], cur[1], cur[2], cur[3])
            for em in g0_ems[:4]:
                em()
            carry = g0_ems[4:]
            pending_y = []
            pending2 = []  # two-deep attention software pipeline (AV+norm
            # of att k runs after scores/exp/mask of att k+2 are emitted)

            def flush_y():
                while pending_y:
                    b, otp_t = pending_y.pop(0)
                    emit_yproj(b, otp_t)

            def run_pending2(keep=0):
                while len(pending2) > keep:
                    pending2.pop(0)()

            for g in range(NG):
                nxt = None
                nxt_ems = list(carry)
                carry = []
                if g + 1 < NG:
                    nxt = load_group(g + 1)
                    nxt_ems += proj_emitters(nxt[0], nxt[1], nxt[2], nxt[3])
                _, qt, kt, vs = cur
                ei = [0]

                def filler(nxt_ems=nxt_ems, ei=ei):
                    # one projection chunk, emitted inside the softmax wait
                    # so the PE always has independent work
                    if ei[0] < len(nxt_ems):
                        nxt_ems[ei[0]]()
                        ei[0] += 1

                for bi in range(GB):
                    b = g * GB + bi
                    o_sbt = osb.tile([128, 2, C], BF16)
                    for pair in range(3):
                        filler()
                        p_t = att_stage1(qt, kt, bi, pair)
                        # the AV+norm of the att two pairs back lands here,
                        # giving the PE ready work while exp/mask run
                        run_pending2(keep=1)
                        filler()
                        if pair == 2:
                            flush_y()

                        def stage2(p_t=p_t, v=vs[bi], pair=pair,
                                   o_sbt=o_sbt, b=b):
                            att_stage2(p_t, v, pair, o_sbt)
                            if pair == 2:
                                pending_y.append((b, emit_trans(o_sbt)))
                        pending2.append(stage2)
                while ei[0] < len(nxt_ems):
                    nxt_ems[ei[0]]()
                    ei[0] += 1
                cur = nxt
            run_pending2(keep=0)
            flush_y()
    return nc


_NC = None


def _get_nc():
    global _NC
    if _NC is None:
        _NC = split_multi_waits(build_kernel())
    return _NC


def kernel(x, Wq, Wk, Wv, Wo, _trace=False):
    bf16 = ml_dtypes.bfloat16
    wq_t = np.ascontiguousarray(Wq.T).astype(bf16)
    wk_t = np.ascontiguousarray(Wk.T).astype(bf16)
    wv_t = np.ascontiguousarray(Wv.T).astype(bf16)
    wo_t = np.ascontiguousarray(Wo.T).astype(bf16)
    in_maps = []
    for i in range(N_CORES):
        xs = x[i * BL : (i + 1) * BL]  # [BL, T, C]
        xs_t = np.ascontiguousarray(xs.transpose(0, 2, 1)).astype(bf16)
        in_maps.append(
            {"xT": xs_t, "wqt": wq_t, "wkt": wk_t, "wvt": wv_t, "wot": wo_t}
        )
    res = run_bass_kernel_spmd(
        _get_nc(), in_maps, list(range(N_CORES)), trace=_trace
    )
    out = np.concatenate([r["y"] for r in res.results], axis=0)
    if _trace:
        return out.astype(np.float32), res
    return out.astype(np.float32)
